# revision 2
# baseline (speedup 1.0000x reference)
"""Trainium2 Bass kernel for GQA attention block (B=2, S=2048, HID=4096, 32Q/8KV heads).

Sharding: hybrid TP4 x DP2 over 8 NeuronCores.
  core c: batch b = c // 4, TP slice t = c % 4.
  Each core handles one batch element, 8 Q heads (2 KV heads): q/k/v projection
  output dim sharded; o_proj input dim sharded -> partial outputs summed on host.

On-chip dataflow is feature-major ("transposed"): qT/kT [head_dim, tokens] so that
RoPE's rotate_half is a 128x128 matmul, attention scores come out keys-major
(softmax sum over partition via ones-matmul), and attn output lands directly in
the layout o_proj needs as lhsT. Matmuls run in float32r (full PE rate at N>=256,
~1e-3 relative precision).
"""
import os
import sys

for _p in ("/opt/trn_rl_repo", "/root/.axon_site"):
    if _p not in sys.path and os.path.isdir(_p):
        sys.path.append(_p)

import numpy as np

B, S_FULL, HID = 2, 2048, 4096
NH, NKV, HD = 32, 8, 128
TP = 4                 # tensor-parallel ways
QH = NH // TP          # 8 q heads per core
KVH = NKV // TP        # 2 kv heads per core
FQ = QH * HD           # 1024
FKV = KVH * HD         # 256
KH = HID // 128        # 32 contraction tiles
SCALE = 1.0 / float(np.sqrt(HD))

last_exec_time_ns = None


def build_nc(S: int = S_FULL):
    """Build the per-core Bass program (SPMD: same program, per-core inputs)."""
    import concourse.bass as bass
    import concourse.tile as tile
    from concourse import bacc, mybir
    from contextlib import ExitStack

    f32, f32r = mybir.dt.float32, mybir.dt.float32r
    TB = min(1024, S)          # token block for projection passes
    QB = min(512, S)           # query block in attention
    NQB = S // QB
    NTB = S // TB
    KT = S // 128              # key tiles
    NB5 = TB // 512 if TB >= 512 else 1   # 512-wide chunks per TB

    nc = bacc.Bacc("TRN2", target_bir_lowering=False, debug=False)

    hsT = nc.dram_tensor("hsT", [HID, S], f32, kind="ExternalInput")
    cosT = nc.dram_tensor("cosT", [HD, S], f32, kind="ExternalInput")
    sinT = nc.dram_tensor("sinT", [HD, S], f32, kind="ExternalInput")
    wq = nc.dram_tensor("wq", [HID, FQ], f32, kind="ExternalInput")
    bq = nc.dram_tensor("bq", [QH, HD], f32, kind="ExternalInput")
    wk = nc.dram_tensor("wk", [HID, FKV], f32, kind="ExternalInput")
    bk = nc.dram_tensor("bk", [KVH, HD], f32, kind="ExternalInput")
    wv = nc.dram_tensor("wv", [HID, FKV], f32, kind="ExternalInput")
    bv = nc.dram_tensor("bv", [KVH, HD], f32, kind="ExternalInput")
    wo = nc.dram_tensor("wo", [FQ, HID], f32, kind="ExternalInput")
    rt = nc.dram_tensor("rt", [HD, HD], f32, kind="ExternalInput")      # R^T for rotate_half
    ident = nc.dram_tensor("ident", [128, 128], f32, kind="ExternalInput")
    ones = nc.dram_tensor("ones", [128, 128], f32, kind="ExternalInput")
    out = nc.dram_tensor("out", [S, HID], f32, kind="ExternalOutput")

    with tile.TileContext(nc) as tc, ExitStack() as ctx:
        Exp = mybir.ActivationFunctionType.Exp

        const = ctx.enter_context(tc.tile_pool(name="const", bufs=1))
        rt_t = const.tile([128, 128], f32r)
        nc.sync.dma_start(rt_t[:], rt.ap().bitcast(f32r))
        id_t = const.tile([128, 128], f32)
        nc.sync.dma_start(id_t[:], ident.ap())
        ones_t = const.tile([128, 128], f32r)
        nc.sync.dma_start(ones_t[:], ones.ap().bitcast(f32r))
        bq_t = const.tile([128, QH], f32)
        nc.sync.dma_start(bq_t[:], bq.ap().rearrange("h p -> p h"))
        bk_t = const.tile([128, KVH], f32)
        nc.sync.dma_start(bk_t[:], bk.ap().rearrange("h p -> p h"))
        bv_t = const.tile([128, KVH], f32)
        nc.sync.dma_start(bv_t[:], bv.ap().rearrange("h p -> p h"))
        cos_t = const.tile([128, S], f32)
        nc.sync.dma_start(cos_t[:], cosT.ap())
        sin_t = const.tile([128, S], f32)
        nc.sync.dma_start(sin_t[:], sinT.ap())

        # Persistent activations (feature-major). attnT overwrites q in place.
        qpool = ctx.enter_context(tc.tile_pool(name="qpool", bufs=1))
        q_t = [qpool.tile([128, S], f32r, name=f"q{h}") for h in range(QH)]
        kvpool = ctx.enter_context(tc.tile_pool(name="kvpool", bufs=1))
        k_t = [kvpool.tile([128, S], f32r, name=f"k{f}") for f in range(KVH)]
        v_t = kvpool.tile([128, KT * FKV], f32r, name="v")  # [tok%128, (kt, kv*128+d)]

        # ---- Phase 1: K/V projections (feature-major) ----
        with tc.tile_pool(name="vTp", bufs=1) as vTp:
            vT_t = [vTp.tile([128, S], f32, name=f"vT{f}") for f in range(KVH)]
            with (
                tc.tile_pool(name="st1", bufs=3) as st1,
                tc.tile_pool(name="pkv", bufs=1, space="PSUM") as pkv,
            ):
                for tb in range(NTB):
                    psk = [pkv.tile([128, TB], f32, name=f"psk_{tb}_{f}", tag=f"psk{f}")
                           for f in range(KVH)]
                    psv = [pkv.tile([128, TB], f32, name=f"psv_{tb}_{f}", tag=f"psv{f}")
                           for f in range(KVH)]
                    for k in range(KH):
                        hs_s = st1.tile([128, TB], f32r, name=f"hs_{tb}_{k}", tag="hs")
                        nc.sync.dma_start(
                            hs_s[:],
                            hsT.ap()[k * 128:(k + 1) * 128, tb * TB:(tb + 1) * TB].bitcast(f32r))
                        wk_s = st1.tile([128, FKV], f32r, name=f"wk_{tb}_{k}", tag="wk")
                        nc.sync.dma_start(wk_s[:], wk.ap()[k * 128:(k + 1) * 128, :].bitcast(f32r))
                        wv_s = st1.tile([128, FKV], f32r, name=f"wv_{tb}_{k}", tag="wv")
                        nc.sync.dma_start(wv_s[:], wv.ap()[k * 128:(k + 1) * 128, :].bitcast(f32r))
                        for f in range(KVH):
                            for nb in range(NB5):
                                sl = slice(nb * 512, min((nb + 1) * 512, TB))
                                nc.tensor.matmul(psk[f][:, sl], wk_s[:, f * 128:(f + 1) * 128],
                                                 hs_s[:, sl], start=(k == 0), stop=(k == KH - 1))
                                nc.tensor.matmul(psv[f][:, sl], wv_s[:, f * 128:(f + 1) * 128],
                                                 hs_s[:, sl], start=(k == 0), stop=(k == KH - 1))
                    for f in range(KVH):
                        tsl = slice(tb * TB, (tb + 1) * TB)
                        nc.vector.tensor_scalar_add(k_t[f][:, tsl], psk[f][:], bk_t[:, f:f + 1])
                        nc.vector.tensor_scalar_add(vT_t[f][:, tsl], psv[f][:], bv_t[:, f:f + 1])

            # ---- Phase 2a: RoPE on K; 2b: transpose vT -> v (token-major) ----
            with (
                tc.tile_pool(name="tmp1", bufs=2) as tmp1,
                tc.tile_pool(name="pr1", bufs=1, space="PSUM") as pr1,
            ):
                for f in range(KVH):
                    for qb in range(NQB):
                        sl = slice(qb * QB, (qb + 1) * QB)
                        psr = pr1.tile([128, QB], f32, name=f"psrk_{f}_{qb}", tag="rope", bufs=2)
                        nc.tensor.matmul(psr[:], rt_t[:], k_t[f][:, sl], start=True, stop=True)
                        t1 = tmp1.tile([128, QB], f32, name=f"t1k_{f}_{qb}", tag="t1")
                        nc.vector.tensor_mul(t1[:], psr[:], sin_t[:, sl])
                        nc.vector.tensor_mul(k_t[f][:, sl], k_t[f][:, sl], cos_t[:, sl])
                        nc.vector.tensor_add(k_t[f][:, sl], k_t[f][:, sl], t1[:])
                for f in range(KVH):
                    for kt in range(KT):
                        pst = pr1.tile([128, 128], f32, name=f"pst_{f}_{kt}", tag="tp", bufs=2)
                        nc.tensor.transpose(pst[:], vT_t[f][:, kt * 128:(kt + 1) * 128], id_t[:])
                        nc.vector.tensor_copy(
                            v_t[:, kt * FKV + f * 128: kt * FKV + (f + 1) * 128], pst[:])

        # ---- Phase 3: Q projection (feature-major), 4 heads x TB per pass ----
        FC = max(1, QH // 4)  # chunks of up to 4 heads
        HPC = QH // FC
        for fc in range(FC):
            with (
                tc.tile_pool(name=f"st2_{fc}", bufs=3) as st2,
                tc.tile_pool(name=f"pq_{fc}", bufs=1, space="PSUM") as pq,
            ):
                for tb in range(NTB):
                    psq = [pq.tile([128, TB], f32, name=f"psq_{fc}_{tb}_{i}", tag=f"psq{i}")
                           for i in range(HPC)]
                    for k in range(KH):
                        hs_s = st2.tile([128, TB], f32r, name=f"hsq_{fc}_{tb}_{k}", tag="hs")
                        nc.sync.dma_start(
                            hs_s[:],
                            hsT.ap()[k * 128:(k + 1) * 128, tb * TB:(tb + 1) * TB].bitcast(f32r))
                        wq_s = st2.tile([128, HPC * 128], f32r, name=f"wq_{fc}_{tb}_{k}", tag="wq")
                        nc.sync.dma_start(
                            wq_s[:],
                            wq.ap()[k * 128:(k + 1) * 128,
                                    fc * HPC * 128:(fc + 1) * HPC * 128].bitcast(f32r))
                        for i in range(HPC):
                            for nb in range(NB5):
                                sl = slice(nb * 512, min((nb + 1) * 512, TB))
                                nc.tensor.matmul(psq[i][:, sl], wq_s[:, i * 128:(i + 1) * 128],
                                                 hs_s[:, sl], start=(k == 0), stop=(k == KH - 1))
                    for i in range(HPC):
                        h = fc * HPC + i
                        tsl = slice(tb * TB, (tb + 1) * TB)
                        nc.vector.tensor_scalar_add(q_t[h][:, tsl], psq[i][:], bq_t[:, h:h + 1])
            # RoPE on this chunk's heads
            with (
                tc.tile_pool(name=f"tmp2_{fc}", bufs=2) as tmp2,
                tc.tile_pool(name=f"pr2_{fc}", bufs=1, space="PSUM") as pr2,
            ):
                for i in range(HPC):
                    h = fc * HPC + i
                    for qb in range(NQB):
                        sl = slice(qb * QB, (qb + 1) * QB)
                        psr = pr2.tile([128, QB], f32, name=f"psrq_{h}_{qb}", tag="rope", bufs=2)
                        nc.tensor.matmul(psr[:], rt_t[:], q_t[h][:, sl], start=True, stop=True)
                        t1 = tmp2.tile([128, QB], f32, name=f"t1q_{h}_{qb}", tag="t1")
                        nc.vector.tensor_mul(t1[:], psr[:], sin_t[:, sl])
                        nc.vector.tensor_mul(q_t[h][:, sl], q_t[h][:, sl], cos_t[:, sl])
                        nc.vector.tensor_add(q_t[h][:, sl], q_t[h][:, sl], t1[:])

        # ---- Phase 4: attention per (head, query block) ----
        with (
            tc.tile_pool(name="expp", bufs=2) as expp,
            tc.tile_pool(name="spool", bufs=2) as spool,
            tc.tile_pool(name="invp", bufs=2) as invp,
            tc.tile_pool(name="pss", bufs=1, space="PSUM") as pss,
            tc.tile_pool(name="pso", bufs=1, space="PSUM") as pso,
            tc.tile_pool(name="psb", bufs=1, space="PSUM") as psb,
        ):
            for h in range(QH):
                f = h // (QH // KVH)  # local kv head (GQA group of 4)
                for qb in range(NQB):
                    sl = slice(qb * QB, (qb + 1) * QB)
                    po = pso.tile([128, QB], f32, name=f"po_{h}_{qb}", tag="oo", bufs=2)
                    ssum = spool.tile([128, QB], f32r, name=f"S_{h}_{qb}", tag="S")
                    ets = []
                    for kt in range(KT):
                        ps = pss.tile([128, QB], f32, name=f"ps_{h}_{qb}_{kt}", tag="ss", bufs=4)
                        nc.tensor.matmul(ps[:], k_t[f][:, kt * 128:(kt + 1) * 128],
                                         q_t[h][:, sl], start=True, stop=True)
                        et = expp.tile([128, QB], f32r, name=f"e_{h}_{qb}_{kt}", tag=f"e{kt}")
                        nc.scalar.activation(et[:], ps[:], Exp, scale=SCALE)
                        ets.append(et)
                        nc.tensor.matmul(po[:], v_t[:, kt * FKV + f * 128: kt * FKV + (f + 1) * 128],
                                         et[:], start=(kt == 0), stop=(kt == KT - 1))
                        if kt == 1:
                            nc.vector.tensor_add(ssum[:], ets[0][:], ets[1][:])
                        elif kt > 1:
                            nc.vector.tensor_add(ssum[:], ssum[:], et[:])
                    if KT == 1:
                        nc.vector.tensor_copy(ssum[:], ets[0][:])
                    pb = psb.tile([128, QB], f32, name=f"pb_{h}_{qb}", tag="bb", bufs=2)
                    nc.tensor.matmul(pb[:], ones_t[:], ssum[:], start=True, stop=True)
                    inv = invp.tile([128, QB], f32, name=f"inv_{h}_{qb}", tag="inv")
                    nc.vector.reciprocal(inv[:], pb[:])
                    # normalized attn output, overwrites q head in place (feature-major)
                    nc.vector.tensor_mul(q_t[h][:, sl], po[:], inv[:])

        # ---- Phase 5: output projection (partial; host sums over TP) ----
        with (
            tc.tile_pool(name="st3", bufs=4) as st3,
            tc.tile_pool(name="osb", bufs=4) as osb,
            tc.tile_pool(name="po5", bufs=1, space="PSUM") as po5,
        ):
            NHB = HID // 512
            for hb in range(NHB):
                wo_s = []
                for fh in range(QH):
                    w = st3.tile([128, 512], f32r, name=f"wo_{hb}_{fh}", tag=f"wo{fh}", bufs=2)
                    nc.sync.dma_start(
                        w[:], wo.ap()[fh * 128:(fh + 1) * 128,
                                      hb * 512:(hb + 1) * 512].bitcast(f32r))
                    wo_s.append(w)
                for tt in range(KT):
                    pot = po5.tile([128, 512], f32, name=f"pot_{hb}_{tt}", tag="po", bufs=4)
                    for fh in range(QH):
                        nc.tensor.matmul(pot[:], q_t[fh][:, tt * 128:(tt + 1) * 128],
                                         wo_s[fh][:], start=(fh == 0), stop=(fh == QH - 1))
                    ot = osb.tile([128, 512], f32, name=f"ot_{hb}_{tt}", tag="ot")
                    nc.scalar.copy(ot[:], pot[:])
                    nc.sync.dma_start(
                        out.ap()[tt * 128:(tt + 1) * 128, hb * 512:(hb + 1) * 512], ot[:])

    nc.compile()
    return nc


def make_host_constants():
    rt = np.zeros((HD, HD), dtype=np.float32)
    half = HD // 2
    for j in range(half):
        rt[j + half, j] = -1.0   # RT[i, i-64] = -1 for i >= 64
        rt[j, j + half] = 1.0    # RT[i, i+64] = +1 for i < 64
    ident = np.eye(128, dtype=np.float32)
    ones = np.ones((128, 128), dtype=np.float32)
    return rt, ident, ones


def shard_inputs(hidden_states, cos, sin, Wq, bq, Wk, bk, Wv, bv, Wo, S=S_FULL):
    rt, ident, ones = make_host_constants()
    in_maps = []
    for c in range(8):
        b, t = c // TP, c % TP
        m = {
            "hsT": np.ascontiguousarray(hidden_states[b].T),
            "cosT": np.ascontiguousarray(cos[b].T),
            "sinT": np.ascontiguousarray(sin[b].T),
            "wq": np.ascontiguousarray(Wq[:, t * FQ:(t + 1) * FQ]),
            "bq": np.ascontiguousarray(bq[t * FQ:(t + 1) * FQ].reshape(QH, HD)),
            "wk": np.ascontiguousarray(Wk[:, t * FKV:(t + 1) * FKV]),
            "bk": np.ascontiguousarray(bk[t * FKV:(t + 1) * FKV].reshape(KVH, HD)),
            "wv": np.ascontiguousarray(Wv[:, t * FKV:(t + 1) * FKV]),
            "bv": np.ascontiguousarray(bv[t * FKV:(t + 1) * FKV].reshape(KVH, HD)),
            "wo": np.ascontiguousarray(Wo[t * FQ:(t + 1) * FQ, :]),
            "rt": rt, "ident": ident, "ones": ones,
        }
        in_maps.append(m)
    return in_maps


_nc_cache = {}


def kernel(hidden_states, cos, sin, Wq, bq, Wk, bk, Wv, bv, Wo):
    global last_exec_time_ns
    from concourse.bass_utils import run_bass_kernel_spmd

    hidden_states = np.asarray(hidden_states, dtype=np.float32)
    cos = np.asarray(cos, dtype=np.float32)
    sin = np.asarray(sin, dtype=np.float32)
    S = hidden_states.shape[1]
    if S not in _nc_cache:
        _nc_cache[S] = build_nc(S)
    nc = _nc_cache[S]
    in_maps = shard_inputs(hidden_states, cos, sin,
                           np.asarray(Wq, np.float32), np.asarray(bq, np.float32),
                           np.asarray(Wk, np.float32), np.asarray(bk, np.float32),
                           np.asarray(Wv, np.float32), np.asarray(bv, np.float32),
                           np.asarray(Wo, np.float32), S=S)
    trace = bool(int(os.environ.get("ATTN_TRACE", "0")))
    r = run_bass_kernel_spmd(nc, in_maps, list(range(8)), trace=trace)
    last_exec_time_ns = r.exec_time_ns
    outs = [r.results[c]["out"] for c in range(8)]
    full = np.empty((B, S, HID), dtype=np.float32)
    for b in range(B):
        full[b] = outs[b * TP]
        for t in range(1, TP):
            full[b] += outs[b * TP + t]
    return full


# revision 5
# speedup vs baseline: 1.0726x; 1.0726x over previous
"""Trainium2 Bass kernel for GQA attention block (B=2, S=2048, HID=4096, 32Q/8KV heads).

Sharding: hybrid TP4 x DP2 over 8 NeuronCores.
  core c: batch b = c // 4, TP slice t = c % 4.
  Each core handles one batch element, 8 Q heads (2 KV heads): q/k/v projection
  output dim sharded; o_proj input dim sharded -> partial outputs summed on host.

On-chip dataflow is feature-major ("transposed"): qT/kT [head_dim, tokens] so that
RoPE's rotate_half is a 128x128 matmul, attention scores come out keys-major
(softmax sum over partition via ones-matmul), and attn output lands directly in
the layout o_proj needs as lhsT. Matmuls run in float32r (full PE rate at N>=256,
~1e-3 relative precision). PSUM evacuations ride on ScalarE; the softmax key-sum
tree is split GpSimd (leaf pairs) / VectorE (upper levels) to keep DVE off the
critical path.
"""
import os
import sys

for _p in ("/opt/trn_rl_repo", "/root/.axon_site"):
    if _p not in sys.path and os.path.isdir(_p):
        sys.path.append(_p)

import numpy as np

B, S_FULL, HID = 2, 2048, 4096
NH, NKV, HD = 32, 8, 128
TP = 4                 # tensor-parallel ways
QH = NH // TP          # 8 q heads per core
KVH = NKV // TP        # 2 kv heads per core
FQ = QH * HD           # 1024
FKV = KVH * HD         # 256
KH = HID // 128        # 32 contraction tiles
SCALE = 1.0 / float(np.sqrt(HD))

last_exec_time_ns = None


def build_nc(S: int = S_FULL):
    """Build the per-core Bass program (SPMD: same program, per-core inputs)."""
    import concourse.bass as bass
    import concourse.tile as tile
    from concourse import bacc, mybir
    from contextlib import ExitStack

    f32, f32r = mybir.dt.float32, mybir.dt.float32r
    TB = min(1024, S)          # token block for projection passes
    QB = min(512, S)           # query block in attention
    NQB = S // QB
    NTB = S // TB
    KT = S // 128              # key tiles
    NB5 = TB // 512 if TB >= 512 else 1   # 512-wide chunks per TB

    nc = bacc.Bacc("TRN2", target_bir_lowering=False, debug=False)

    hsT = nc.dram_tensor("hsT", [HID, S], f32, kind="ExternalInput")
    cosT = nc.dram_tensor("cosT", [HD, S], f32, kind="ExternalInput")
    sinT = nc.dram_tensor("sinT", [HD, S], f32, kind="ExternalInput")
    wq = nc.dram_tensor("wq", [HID, FQ], f32, kind="ExternalInput")
    bq = nc.dram_tensor("bq", [QH, HD], f32, kind="ExternalInput")
    wk = nc.dram_tensor("wk", [HID, FKV], f32, kind="ExternalInput")
    bk = nc.dram_tensor("bk", [KVH, HD], f32, kind="ExternalInput")
    wv = nc.dram_tensor("wv", [HID, FKV], f32, kind="ExternalInput")
    bv = nc.dram_tensor("bv", [KVH, HD], f32, kind="ExternalInput")
    wo = nc.dram_tensor("wo", [FQ, HID], f32, kind="ExternalInput")
    rt = nc.dram_tensor("rt", [HD, HD], f32, kind="ExternalInput")      # R^T for rotate_half
    ident = nc.dram_tensor("ident", [128, 128], f32, kind="ExternalInput")
    ones = nc.dram_tensor("ones", [128, 128], f32, kind="ExternalInput")
    out = nc.dram_tensor("out", [S, HID], f32, kind="ExternalOutput")

    with tile.TileContext(nc) as tc, ExitStack() as ctx:
        Exp = mybir.ActivationFunctionType.Exp
        Ident = mybir.ActivationFunctionType.Identity

        const = ctx.enter_context(tc.tile_pool(name="const", bufs=1))
        bq_t = const.tile([128, QH], f32)
        nc.sync.dma_start(bq_t[:], bq.ap().rearrange("h p -> p h"))
        bk_t = const.tile([128, KVH], f32)
        nc.sync.dma_start(bk_t[:], bk.ap().rearrange("h p -> p h"))
        bv_t = const.tile([128, KVH], f32)
        nc.sync.dma_start(bv_t[:], bv.ap().rearrange("h p -> p h"))
        # rope/attention constants traced later (overlap phase-1 DMA), tiles here
        rt_t = const.tile([128, 128], f32r)
        id_t = const.tile([128, 128], f32)
        ones_t = const.tile([128, 128], f32r)
        cos_t = const.tile([128, S], f32)
        sin_t = const.tile([128, S], f32)

        # Persistent activations (feature-major). attnT overwrites q in place.
        qpool = ctx.enter_context(tc.tile_pool(name="qpool", bufs=1))
        q_t = [qpool.tile([128, S], f32r, name=f"q{h}") for h in range(QH)]
        kvpool = ctx.enter_context(tc.tile_pool(name="kvpool", bufs=1))
        k_t = [kvpool.tile([128, S], f32r, name=f"k{f}") for f in range(KVH)]
        v_t = kvpool.tile([128, KT * FKV], f32r, name="v")  # [tok%128, (kt, kv*128+d)]

        def rope_inplace(x_t, sl, psum_pool, sb_pool, tag):
            """x[:, sl] = x[:, sl]*cos + (R @ x[:, sl])*sin, via PE rot + ACT evac."""
            psr = psum_pool.tile([128, QB], f32, name=f"psr_{tag}", tag="rope", bufs=2)
            nc.tensor.matmul(psr[:], rt_t[:], x_t[:, sl], start=True, stop=True)
            rot = sb_pool.tile([128, QB], f32, name=f"rot_{tag}", tag="rot", bufs=3)
            nc.scalar.copy(rot[:], psr[:])
            t1 = sb_pool.tile([128, QB], f32, name=f"t1_{tag}", tag="t1", bufs=3)
            nc.gpsimd.tensor_mul(t1[:], rot[:], sin_t[:, sl])
            nc.vector.tensor_mul(x_t[:, sl], x_t[:, sl], cos_t[:, sl])
            nc.vector.tensor_add(x_t[:, sl], x_t[:, sl], t1[:])

        # ---- Phase 1: K/V projections (feature-major) ----
        with tc.tile_pool(name="vTp", bufs=1) as vTp:
            vT_t = [vTp.tile([128, S], f32, name=f"vT{f}") for f in range(KVH)]
            with (
                tc.tile_pool(name="st1", bufs=6) as st1,
                tc.tile_pool(name="pkv", bufs=1, space="PSUM") as pkv,
            ):
                for tb in range(NTB):
                    psk = [pkv.tile([128, TB], f32, name=f"psk_{tb}_{f}", tag=f"psk{f}")
                           for f in range(KVH)]
                    psv = [pkv.tile([128, TB], f32, name=f"psv_{tb}_{f}", tag=f"psv{f}")
                           for f in range(KVH)]
                    for k in range(KH):
                        hs_s = st1.tile([128, TB], f32r, name=f"hs_{tb}_{k}", tag="hs")
                        nc.sync.dma_start(
                            hs_s[:],
                            hsT.ap()[k * 128:(k + 1) * 128, tb * TB:(tb + 1) * TB].bitcast(f32r))
                        wk_s = st1.tile([128, FKV], f32r, name=f"wk_{tb}_{k}", tag="wk", bufs=4)
                        nc.sync.dma_start(wk_s[:], wk.ap()[k * 128:(k + 1) * 128, :].bitcast(f32r))
                        wv_s = st1.tile([128, FKV], f32r, name=f"wv_{tb}_{k}", tag="wv", bufs=4)
                        nc.sync.dma_start(wv_s[:], wv.ap()[k * 128:(k + 1) * 128, :].bitcast(f32r))
                        for f in range(KVH):
                            for nb in range(NB5):
                                sl = slice(nb * 512, min((nb + 1) * 512, TB))
                                nc.tensor.matmul(psk[f][:, sl], wk_s[:, f * 128:(f + 1) * 128],
                                                 hs_s[:, sl], start=(k == 0), stop=(k == KH - 1))
                                nc.tensor.matmul(psv[f][:, sl], wv_s[:, f * 128:(f + 1) * 128],
                                                 hs_s[:, sl], start=(k == 0), stop=(k == KH - 1))
                    if tb == 0:
                        # rope/attention constants: DMA overlaps the matmul stream
                        nc.sync.dma_start(rt_t[:], rt.ap().bitcast(f32r))
                        nc.sync.dma_start(id_t[:], ident.ap())
                        nc.sync.dma_start(ones_t[:], ones.ap().bitcast(f32r))
                        nc.sync.dma_start(cos_t[:], cosT.ap())
                        nc.sync.dma_start(sin_t[:], sinT.ap())
                    for f in range(KVH):
                        tsl = slice(tb * TB, (tb + 1) * TB)
                        nc.scalar.activation(k_t[f][:, tsl], psk[f][:], Ident,
                                             bias=bk_t[:, f:f + 1])
                        nc.scalar.activation(vT_t[f][:, tsl], psv[f][:], Ident,
                                             bias=bv_t[:, f:f + 1])

            # ---- Phase 2a: RoPE on K; 2b: transpose vT -> v (token-major) ----
            with (
                tc.tile_pool(name="tmp1", bufs=1) as tmp1,
                tc.tile_pool(name="pr1", bufs=1, space="PSUM") as pr1,
            ):
                for f in range(KVH):
                    for qb in range(NQB):
                        rope_inplace(k_t[f], slice(qb * QB, (qb + 1) * QB),
                                     pr1, tmp1, f"k{f}_{qb}")
                for f in range(KVH):
                    for kt in range(KT):
                        pst = pr1.tile([128, 128], f32, name=f"pst_{f}_{kt}", tag="tp", bufs=2)
                        nc.tensor.transpose(pst[:], vT_t[f][:, kt * 128:(kt + 1) * 128], id_t[:])
                        nc.scalar.copy(
                            v_t[:, kt * FKV + f * 128: kt * FKV + (f + 1) * 128], pst[:])

        # ---- Phase 3: Q projection (feature-major), 4 heads x TB per pass ----
        FC = max(1, QH // 4)  # chunks of up to 4 heads
        HPC = QH // FC
        for fc in range(FC):
            with (
                tc.tile_pool(name=f"st2_{fc}", bufs=6) as st2,
                tc.tile_pool(name=f"pq_{fc}", bufs=1, space="PSUM") as pq,
            ):
                for tb in range(NTB):
                    psq = [pq.tile([128, TB], f32, name=f"psq_{fc}_{tb}_{i}", tag=f"psq{i}")
                           for i in range(HPC)]
                    for k in range(KH):
                        hs_s = st2.tile([128, TB], f32r, name=f"hsq_{fc}_{tb}_{k}", tag="hs")
                        nc.sync.dma_start(
                            hs_s[:],
                            hsT.ap()[k * 128:(k + 1) * 128, tb * TB:(tb + 1) * TB].bitcast(f32r))
                        wq_s = st2.tile([128, HPC * 128], f32r, name=f"wq_{fc}_{tb}_{k}", tag="wq")
                        nc.sync.dma_start(
                            wq_s[:],
                            wq.ap()[k * 128:(k + 1) * 128,
                                    fc * HPC * 128:(fc + 1) * HPC * 128].bitcast(f32r))
                        for i in range(HPC):
                            for nb in range(NB5):
                                sl = slice(nb * 512, min((nb + 1) * 512, TB))
                                nc.tensor.matmul(psq[i][:, sl], wq_s[:, i * 128:(i + 1) * 128],
                                                 hs_s[:, sl], start=(k == 0), stop=(k == KH - 1))
                    for i in range(HPC):
                        h = fc * HPC + i
                        tsl = slice(tb * TB, (tb + 1) * TB)
                        nc.scalar.activation(q_t[h][:, tsl], psq[i][:], Ident,
                                             bias=bq_t[:, h:h + 1])
            # RoPE on this chunk's heads (ACT evac lets it overlap the next pass)
            with (
                tc.tile_pool(name=f"tmp2_{fc}", bufs=1) as tmp2,
                tc.tile_pool(name=f"pr2_{fc}", bufs=1, space="PSUM") as pr2,
            ):
                for i in range(HPC):
                    h = fc * HPC + i
                    for qb in range(NQB):
                        rope_inplace(q_t[h], slice(qb * QB, (qb + 1) * QB),
                                     pr2, tmp2, f"q{h}_{qb}")

        # ---- Phase 4: attention per (head, query block) ----
        with (
            tc.tile_pool(name="expp", bufs=2) as expp,
            tc.tile_pool(name="spool", bufs=2) as spool,
            tc.tile_pool(name="invp", bufs=2) as invp,
            tc.tile_pool(name="pss", bufs=1, space="PSUM") as pss,
            tc.tile_pool(name="pso", bufs=1, space="PSUM") as pso,
            tc.tile_pool(name="psb", bufs=1, space="PSUM") as psb,
        ):
            for h in range(QH):
                f = h // (QH // KVH)  # local kv head (GQA group of 4)
                for qb in range(NQB):
                    sl = slice(qb * QB, (qb + 1) * QB)
                    po = pso.tile([128, QB], f32, name=f"po_{h}_{qb}", tag="oo", bufs=2)
                    prev_et = None
                    level = []
                    for kt in range(KT):
                        ps = pss.tile([128, QB], f32, name=f"ps_{h}_{qb}_{kt}", tag="ss", bufs=5)
                        nc.tensor.matmul(ps[:], k_t[f][:, kt * 128:(kt + 1) * 128],
                                         q_t[h][:, sl], start=True, stop=True)
                        et = expp.tile([128, QB], f32r, name=f"e_{h}_{qb}_{kt}", tag="et", bufs=5)
                        nc.scalar.activation(et[:], ps[:], Exp, scale=SCALE)
                        nc.tensor.matmul(po[:], v_t[:, kt * FKV + f * 128: kt * FKV + (f + 1) * 128],
                                         et[:], start=(kt == 0), stop=(kt == KT - 1))
                        # key-sum tree leaves on GpSimd as pairs complete
                        if kt % 2 == 1:
                            pt = spool.tile([128, QB], f32, name=f"pa_{h}_{qb}_{kt // 2}",
                                            tag=f"pa{kt // 2}", bufs=2)
                            nc.gpsimd.tensor_add(pt[:], prev_et[:], et[:])
                            level.append(pt)
                            prev_et = None
                        else:
                            prev_et = et
                    if prev_et is not None:
                        level.append(prev_et)
                    # upper tree levels on DVE (in place on pair tiles)
                    while len(level) > 2:
                        nxt = []
                        for j in range(len(level) // 2):
                            dst = level[2 * j]
                            nc.vector.tensor_add(dst[:], dst[:], level[2 * j + 1][:])
                            nxt.append(dst)
                        if len(level) % 2:
                            nxt.append(level[-1])
                        level = nxt
                    ssum = spool.tile([128, QB], f32r, name=f"S_{h}_{qb}", tag="S")
                    if len(level) == 2:
                        nc.vector.tensor_add(ssum[:], level[0][:], level[1][:])
                    else:
                        nc.vector.tensor_copy(ssum[:], level[0][:])
                    pb = psb.tile([128, QB], f32, name=f"pb_{h}_{qb}", tag="bb", bufs=1)
                    nc.tensor.matmul(pb[:], ones_t[:], ssum[:], start=True, stop=True)
                    inv = invp.tile([128, QB], f32, name=f"inv_{h}_{qb}", tag="inv")
                    nc.vector.reciprocal_approx_fast(inv[:], pb[:])
                    # normalized attn output, overwrites q head in place (feature-major)
                    nc.vector.tensor_mul(q_t[h][:, sl], po[:], inv[:])

        # ---- Phase 5: output projection (partial; host sums over TP) ----
        with (
            tc.tile_pool(name="st3", bufs=4) as st3,
            tc.tile_pool(name="osb", bufs=4) as osb,
            tc.tile_pool(name="po5", bufs=1, space="PSUM") as po5,
        ):
            NHB = HID // 512
            for hb in range(NHB):
                wo_s = []
                for fh in range(QH):
                    w = st3.tile([128, 512], f32r, name=f"wo_{hb}_{fh}", tag=f"wo{fh}", bufs=2)
                    nc.sync.dma_start(
                        w[:], wo.ap()[fh * 128:(fh + 1) * 128,
                                      hb * 512:(hb + 1) * 512].bitcast(f32r))
                    wo_s.append(w)
                for tt in range(KT):
                    pot = po5.tile([128, 512], f32, name=f"pot_{hb}_{tt}", tag="po", bufs=4)
                    for fh in range(QH):
                        nc.tensor.matmul(pot[:], q_t[fh][:, tt * 128:(tt + 1) * 128],
                                         wo_s[fh][:], start=(fh == 0), stop=(fh == QH - 1))
                    ot = osb.tile([128, 512], f32, name=f"ot_{hb}_{tt}", tag="ot")
                    nc.scalar.copy(ot[:], pot[:])
                    nc.sync.dma_start(
                        out.ap()[tt * 128:(tt + 1) * 128, hb * 512:(hb + 1) * 512], ot[:])

    nc.compile()
    return nc


def make_host_constants():
    rt = np.zeros((HD, HD), dtype=np.float32)
    half = HD // 2
    for j in range(half):
        rt[j + half, j] = -1.0   # RT[i, i-64] = -1 for i >= 64
        rt[j, j + half] = 1.0    # RT[i, i+64] = +1 for i < 64
    ident = np.eye(128, dtype=np.float32)
    ones = np.ones((128, 128), dtype=np.float32)
    return rt, ident, ones


def shard_inputs(hidden_states, cos, sin, Wq, bq, Wk, bk, Wv, bv, Wo, S=S_FULL):
    rt, ident, ones = make_host_constants()
    in_maps = []
    for c in range(8):
        b, t = c // TP, c % TP
        m = {
            "hsT": np.ascontiguousarray(hidden_states[b].T),
            "cosT": np.ascontiguousarray(cos[b].T),
            "sinT": np.ascontiguousarray(sin[b].T),
            "wq": np.ascontiguousarray(Wq[:, t * FQ:(t + 1) * FQ]),
            "bq": np.ascontiguousarray(bq[t * FQ:(t + 1) * FQ].reshape(QH, HD)),
            "wk": np.ascontiguousarray(Wk[:, t * FKV:(t + 1) * FKV]),
            "bk": np.ascontiguousarray(bk[t * FKV:(t + 1) * FKV].reshape(KVH, HD)),
            "wv": np.ascontiguousarray(Wv[:, t * FKV:(t + 1) * FKV]),
            "bv": np.ascontiguousarray(bv[t * FKV:(t + 1) * FKV].reshape(KVH, HD)),
            "wo": np.ascontiguousarray(Wo[t * FQ:(t + 1) * FQ, :]),
            "rt": rt, "ident": ident, "ones": ones,
        }
        in_maps.append(m)
    return in_maps


_nc_cache = {}


def kernel(hidden_states, cos, sin, Wq, bq, Wk, bk, Wv, bv, Wo):
    global last_exec_time_ns
    from concourse.bass_utils import run_bass_kernel_spmd

    hidden_states = np.asarray(hidden_states, dtype=np.float32)
    cos = np.asarray(cos, dtype=np.float32)
    sin = np.asarray(sin, dtype=np.float32)
    S = hidden_states.shape[1]
    if S not in _nc_cache:
        _nc_cache[S] = build_nc(S)
    nc = _nc_cache[S]
    in_maps = shard_inputs(hidden_states, cos, sin,
                           np.asarray(Wq, np.float32), np.asarray(bq, np.float32),
                           np.asarray(Wk, np.float32), np.asarray(bk, np.float32),
                           np.asarray(Wv, np.float32), np.asarray(bv, np.float32),
                           np.asarray(Wo, np.float32), S=S)
    trace = bool(int(os.environ.get("ATTN_TRACE", "0")))
    r = run_bass_kernel_spmd(nc, in_maps, list(range(8)), trace=trace)
    last_exec_time_ns = r.exec_time_ns
    outs = [r.results[c]["out"] for c in range(8)]
    full = np.empty((B, S, HID), dtype=np.float32)
    for b in range(B):
        full[b] = outs[b * TP]
        for t in range(1, TP):
            full[b] += outs[b * TP + t]
    return full


# revision 8
# speedup vs baseline: 1.4237x; 1.3273x over previous
"""Trainium2 Bass kernel for GQA attention block (B=2, S=2048, HID=4096, 32Q/8KV heads).

Sharding: hybrid TP4 x DP2 over 8 NeuronCores.
  core c: batch b = c // 4, TP slice t = c % 4.
  Each core handles one batch element, 8 Q heads (2 KV heads): q/k/v projection
  output dim sharded; o_proj input dim sharded -> partial outputs summed on host.

On-chip dataflow is feature-major ("transposed"): qT/kT [head_dim, tokens] so that
RoPE's rotate_half is a 128x128 matmul, attention scores come out keys-major
(softmax sum over partition via ones-matmul), and attn output lands directly in
the layout o_proj needs as lhsT. Matmuls run in float32r (full PE rate at N>=256,
~1e-3 relative precision). PSUM evacuations ride on ScalarE; the softmax key-sum
tree is split GpSimd (leaf pairs) / VectorE (upper levels) to keep DVE off the
critical path.
"""
import os
import sys

for _p in ("/opt/trn_rl_repo", "/root/.axon_site"):
    if _p not in sys.path and os.path.isdir(_p):
        sys.path.append(_p)

import numpy as np

B, S_FULL, HID = 2, 2048, 4096
NH, NKV, HD = 32, 8, 128
TP = 4                 # tensor-parallel ways
QH = NH // TP          # 8 q heads per core
KVH = NKV // TP        # 2 kv heads per core
FQ = QH * HD           # 1024
FKV = KVH * HD         # 256
KH = HID // 128        # 32 contraction tiles
SCALE = 1.0 / float(np.sqrt(HD))

last_exec_time_ns = None


def build_nc(S: int = S_FULL, dt: str = "f32r"):
    """Build the per-core Bass program (SPMD: same program, per-core inputs)."""
    import concourse.bass as bass
    import concourse.tile as tile
    from concourse import bacc, mybir
    from contextlib import ExitStack

    f32 = mybir.dt.float32
    f32r = mybir.dt.float32r if dt == "f32r" else mybir.dt.bfloat16  # matmul dtype
    dma_dt = f32 if dt == "f32r" else mybir.dt.bfloat16              # big-input DRAM dtype
    csdt = f32 if dt == "f32r" else mybir.dt.bfloat16                # cos/sin dtype
    TB = min(1024, S)          # token block for projection passes
    QB = min(512, S)           # query block in attention
    NQB = S // QB
    NTB = S // TB
    KT = S // 128              # key tiles
    NB5 = TB // 512 if TB >= 512 else 1   # 512-wide chunks per TB

    nc = bacc.Bacc("TRN2", target_bir_lowering=False, debug=False)

    hsT = nc.dram_tensor("hsT", [HID, S], dma_dt, kind="ExternalInput")
    cosT = nc.dram_tensor("cosT", [HD, S], csdt, kind="ExternalInput")
    sinT = nc.dram_tensor("sinT", [HD, S], csdt, kind="ExternalInput")
    wq = nc.dram_tensor("wq", [HID, FQ], dma_dt, kind="ExternalInput")
    bq = nc.dram_tensor("bq", [QH, HD], f32, kind="ExternalInput")
    wk = nc.dram_tensor("wk", [HID, FKV], dma_dt, kind="ExternalInput")
    bk = nc.dram_tensor("bk", [KVH, HD], f32, kind="ExternalInput")
    wv = nc.dram_tensor("wv", [HID, FKV], dma_dt, kind="ExternalInput")
    bv = nc.dram_tensor("bv", [KVH, HD], f32, kind="ExternalInput")
    wo = nc.dram_tensor("wo", [FQ, HID], dma_dt, kind="ExternalInput")
    rt = nc.dram_tensor("rt", [HD, HD], dma_dt, kind="ExternalInput")      # R^T for rotate_half
    ident = nc.dram_tensor("ident", [128, 128], dma_dt, kind="ExternalInput")
    ones = nc.dram_tensor("ones", [128, 128], dma_dt, kind="ExternalInput")
    out = nc.dram_tensor("out", [S, HID], f32, kind="ExternalOutput")

    with tile.TileContext(nc) as tc, ExitStack() as ctx:
        Exp = mybir.ActivationFunctionType.Exp
        Ident = mybir.ActivationFunctionType.Identity

        const = ctx.enter_context(tc.tile_pool(name="const", bufs=1))
        bq_t = const.tile([128, QH], f32)
        nc.sync.dma_start(bq_t[:], bq.ap().rearrange("h p -> p h"))
        bk_t = const.tile([128, KVH], f32)
        nc.sync.dma_start(bk_t[:], bk.ap().rearrange("h p -> p h"))
        bv_t = const.tile([128, KVH], f32)
        nc.sync.dma_start(bv_t[:], bv.ap().rearrange("h p -> p h"))
        # rope/attention constants traced later (overlap phase-1 DMA), tiles here
        rt_t = const.tile([128, 128], f32r)
        id_t = const.tile([128, 128], f32r)
        ones_t = const.tile([128, 128], f32r)
        cos_t = const.tile([128, S], csdt)
        sin_t = const.tile([128, S], csdt)

        # Persistent activations (feature-major). attnT overwrites q in place.
        qpool = ctx.enter_context(tc.tile_pool(name="qpool", bufs=1))
        q_t = [qpool.tile([128, S], f32r, name=f"q{h}") for h in range(QH)]
        kvpool = ctx.enter_context(tc.tile_pool(name="kvpool", bufs=1))
        k_t = [kvpool.tile([128, S], f32r, name=f"k{f}") for f in range(KVH)]
        v_t = kvpool.tile([128, KT * FKV], f32r, name="v")  # [tok%128, (kt, kv*128+d)]

        def rope_inplace(x_t, sl, psum_pool, sb_pool, tag):
            """x[:, sl] = x[:, sl]*cos + (R @ x[:, sl])*sin, via PE rot + ACT evac."""
            psr = psum_pool.tile([128, QB], f32, name=f"psr_{tag}", tag="rope", bufs=2)
            nc.tensor.matmul(psr[:], rt_t[:], x_t[:, sl], start=True, stop=True)
            rot = sb_pool.tile([128, QB], csdt, name=f"rot_{tag}", tag="rot", bufs=3)
            nc.scalar.copy(rot[:], psr[:])
            t1 = sb_pool.tile([128, QB], csdt, name=f"t1_{tag}", tag="t1", bufs=3)
            nc.vector.tensor_mul(t1[:], rot[:], sin_t[:, sl])
            nc.vector.tensor_mul(x_t[:, sl], x_t[:, sl], cos_t[:, sl])
            nc.vector.tensor_add(x_t[:, sl], x_t[:, sl], t1[:])

        # ---- Phase 1: K/V projections (feature-major) ----
        with tc.tile_pool(name="vTp", bufs=1) as vTp:
            vT_t = [vTp.tile([128, S], f32r, name=f"vT{f}") for f in range(KVH)]
            with (
                tc.tile_pool(name="st1", bufs=6) as st1,
                tc.tile_pool(name="pkv", bufs=1, space="PSUM") as pkv,
            ):
                for tb in range(NTB):
                    psk = [pkv.tile([128, TB], f32, name=f"psk_{tb}_{f}", tag=f"psk{f}")
                           for f in range(KVH)]
                    psv = [pkv.tile([128, TB], f32, name=f"psv_{tb}_{f}", tag=f"psv{f}")
                           for f in range(KVH)]
                    for k in range(KH):
                        hs_s = st1.tile([128, TB], f32r, name=f"hs_{tb}_{k}", tag="hs")
                        nc.sync.dma_start(
                            hs_s[:],
                            hsT.ap()[k * 128:(k + 1) * 128, tb * TB:(tb + 1) * TB].bitcast(f32r))
                        wk_s = st1.tile([128, FKV], f32r, name=f"wk_{tb}_{k}", tag="wk", bufs=4)
                        nc.sync.dma_start(wk_s[:], wk.ap()[k * 128:(k + 1) * 128, :].bitcast(f32r))
                        wv_s = st1.tile([128, FKV], f32r, name=f"wv_{tb}_{k}", tag="wv", bufs=4)
                        nc.sync.dma_start(wv_s[:], wv.ap()[k * 128:(k + 1) * 128, :].bitcast(f32r))
                        for f in range(KVH):
                            for nb in range(NB5):
                                sl = slice(nb * 512, min((nb + 1) * 512, TB))
                                nc.tensor.matmul(psk[f][:, sl], wk_s[:, f * 128:(f + 1) * 128],
                                                 hs_s[:, sl], start=(k == 0), stop=(k == KH - 1))
                                nc.tensor.matmul(psv[f][:, sl], wv_s[:, f * 128:(f + 1) * 128],
                                                 hs_s[:, sl], start=(k == 0), stop=(k == KH - 1))
                    if tb == 0:
                        # rope/attention constants: DMA overlaps the matmul stream
                        nc.sync.dma_start(rt_t[:], rt.ap().bitcast(f32r))
                        nc.sync.dma_start(id_t[:], ident.ap().bitcast(f32r))
                        nc.sync.dma_start(ones_t[:], ones.ap().bitcast(f32r))
                        nc.sync.dma_start(cos_t[:], cosT.ap())
                        nc.sync.dma_start(sin_t[:], sinT.ap())
                    for f in range(KVH):
                        tsl = slice(tb * TB, (tb + 1) * TB)
                        nc.scalar.activation(k_t[f][:, tsl], psk[f][:], Ident,
                                             bias=bk_t[:, f:f + 1])
                        nc.scalar.activation(vT_t[f][:, tsl], psv[f][:], Ident,
                                             bias=bv_t[:, f:f + 1])

            # ---- Phase 2a: RoPE on K; 2b: transpose vT -> v (token-major) ----
            with (
                tc.tile_pool(name="tmp1", bufs=1) as tmp1,
                tc.tile_pool(name="pr1", bufs=1, space="PSUM") as pr1,
            ):
                for f in range(KVH):
                    for qb in range(NQB):
                        rope_inplace(k_t[f], slice(qb * QB, (qb + 1) * QB),
                                     pr1, tmp1, f"k{f}_{qb}")
                for f in range(KVH):
                    for kt in range(KT):
                        pst = pr1.tile([128, 128], f32r, name=f"pst_{f}_{kt}", tag="tp", bufs=2)
                        nc.tensor.transpose(pst[:], vT_t[f][:, kt * 128:(kt + 1) * 128], id_t[:])
                        nc.scalar.copy(
                            v_t[:, kt * FKV + f * 128: kt * FKV + (f + 1) * 128], pst[:])

        # ---- Phase 3: Q projection (feature-major), 4 heads x TB per pass ----
        FC = max(1, QH // 4)  # chunks of up to 4 heads
        HPC = QH // FC
        for fc in range(FC):
            with (
                tc.tile_pool(name=f"st2_{fc}", bufs=6) as st2,
                tc.tile_pool(name=f"pq_{fc}", bufs=1, space="PSUM") as pq,
            ):
                for tb in range(NTB):
                    psq = [pq.tile([128, TB], f32, name=f"psq_{fc}_{tb}_{i}", tag=f"psq{i}")
                           for i in range(HPC)]
                    for k in range(KH):
                        hs_s = st2.tile([128, TB], f32r, name=f"hsq_{fc}_{tb}_{k}", tag="hs")
                        nc.sync.dma_start(
                            hs_s[:],
                            hsT.ap()[k * 128:(k + 1) * 128, tb * TB:(tb + 1) * TB].bitcast(f32r))
                        wq_s = st2.tile([128, HPC * 128], f32r, name=f"wq_{fc}_{tb}_{k}", tag="wq")
                        nc.sync.dma_start(
                            wq_s[:],
                            wq.ap()[k * 128:(k + 1) * 128,
                                    fc * HPC * 128:(fc + 1) * HPC * 128].bitcast(f32r))
                        for i in range(HPC):
                            for nb in range(NB5):
                                sl = slice(nb * 512, min((nb + 1) * 512, TB))
                                nc.tensor.matmul(psq[i][:, sl], wq_s[:, i * 128:(i + 1) * 128],
                                                 hs_s[:, sl], start=(k == 0), stop=(k == KH - 1))
                    for i in range(HPC):
                        h = fc * HPC + i
                        tsl = slice(tb * TB, (tb + 1) * TB)
                        nc.scalar.activation(q_t[h][:, tsl], psq[i][:], Ident,
                                             bias=bq_t[:, h:h + 1])
            # RoPE on this chunk's heads (ACT evac lets it overlap the next pass)
            with (
                tc.tile_pool(name=f"tmp2_{fc}", bufs=1) as tmp2,
                tc.tile_pool(name=f"pr2_{fc}", bufs=1, space="PSUM") as pr2,
            ):
                for i in range(HPC):
                    h = fc * HPC + i
                    for qb in range(NQB):
                        rope_inplace(q_t[h], slice(qb * QB, (qb + 1) * QB),
                                     pr2, tmp2, f"q{h}_{qb}")

        # ---- Phase 4: attention per (head, query block) ----
        with (
            tc.tile_pool(name="expp", bufs=2) as expp,
            tc.tile_pool(name="spool", bufs=2) as spool,
            tc.tile_pool(name="invp", bufs=2) as invp,
            tc.tile_pool(name="pss", bufs=1, space="PSUM") as pss,
            tc.tile_pool(name="pso", bufs=1, space="PSUM") as pso,
            tc.tile_pool(name="psb", bufs=1, space="PSUM") as psb,
        ):
            for h in range(QH):
                f = h // (QH // KVH)  # local kv head (GQA group of 4)
                for qb in range(NQB):
                    sl = slice(qb * QB, (qb + 1) * QB)
                    po = pso.tile([128, QB], f32, name=f"po_{h}_{qb}", tag="oo", bufs=2)
                    prev_et = None
                    level = []
                    for kt in range(KT):
                        ps = pss.tile([128, QB], f32, name=f"ps_{h}_{qb}_{kt}", tag="ss", bufs=5)
                        nc.tensor.matmul(ps[:], k_t[f][:, kt * 128:(kt + 1) * 128],
                                         q_t[h][:, sl], start=True, stop=True)
                        et = expp.tile([128, QB], f32r, name=f"e_{h}_{qb}_{kt}", tag="et", bufs=5)
                        nc.scalar.activation(et[:], ps[:], Exp, scale=SCALE)
                        nc.tensor.matmul(po[:], v_t[:, kt * FKV + f * 128: kt * FKV + (f + 1) * 128],
                                         et[:], start=(kt == 0), stop=(kt == KT - 1))
                        # key-sum tree leaves on GpSimd as pairs complete
                        if kt % 2 == 1:
                            pt = spool.tile([128, QB], csdt, name=f"pa_{h}_{qb}_{kt // 2}",
                                            tag=f"pa{kt // 2}", bufs=2)
                            nc.vector.tensor_add(pt[:], prev_et[:], et[:])
                            level.append(pt)
                            prev_et = None
                        else:
                            prev_et = et
                    if prev_et is not None:
                        level.append(prev_et)
                    # upper tree levels on DVE (in place on pair tiles)
                    while len(level) > 2:
                        nxt = []
                        for j in range(len(level) // 2):
                            dst = level[2 * j]
                            nc.vector.tensor_add(dst[:], dst[:], level[2 * j + 1][:])
                            nxt.append(dst)
                        if len(level) % 2:
                            nxt.append(level[-1])
                        level = nxt
                    ssum = spool.tile([128, QB], f32r, name=f"S_{h}_{qb}", tag="S")
                    if len(level) == 2:
                        nc.vector.tensor_add(ssum[:], level[0][:], level[1][:])
                    else:
                        nc.vector.tensor_copy(ssum[:], level[0][:])
                    pb = psb.tile([128, QB], f32, name=f"pb_{h}_{qb}", tag="bb", bufs=1)
                    nc.tensor.matmul(pb[:], ones_t[:], ssum[:], start=True, stop=True)
                    inv = invp.tile([128, QB], f32, name=f"inv_{h}_{qb}", tag="inv")
                    nc.vector.reciprocal_approx_fast(inv[:], pb[:])
                    # normalized attn output, overwrites q head in place (feature-major)
                    nc.vector.tensor_mul(q_t[h][:, sl], po[:], inv[:])

        # ---- Phase 5: output projection (partial; host sums over TP) ----
        with (
            tc.tile_pool(name="st3", bufs=4) as st3,
            tc.tile_pool(name="osb", bufs=4) as osb,
            tc.tile_pool(name="po5", bufs=1, space="PSUM") as po5,
        ):
            NHB = HID // 512
            for hb in range(NHB):
                wo_s = []
                for fh in range(QH):
                    w = st3.tile([128, 512], f32r, name=f"wo_{hb}_{fh}", tag=f"wo{fh}", bufs=2)
                    nc.sync.dma_start(
                        w[:], wo.ap()[fh * 128:(fh + 1) * 128,
                                      hb * 512:(hb + 1) * 512].bitcast(f32r))
                    wo_s.append(w)
                for tt in range(KT):
                    pot = po5.tile([128, 512], f32, name=f"pot_{hb}_{tt}", tag="po", bufs=4)
                    for fh in range(QH):
                        nc.tensor.matmul(pot[:], q_t[fh][:, tt * 128:(tt + 1) * 128],
                                         wo_s[fh][:], start=(fh == 0), stop=(fh == QH - 1))
                    ot = osb.tile([128, 512], f32, name=f"ot_{hb}_{tt}", tag="ot")
                    nc.scalar.copy(ot[:], pot[:])
                    nc.sync.dma_start(
                        out.ap()[tt * 128:(tt + 1) * 128, hb * 512:(hb + 1) * 512], ot[:])

    nc.compile()
    return nc


def make_host_constants():
    rt = np.zeros((HD, HD), dtype=np.float32)
    half = HD // 2
    for j in range(half):
        rt[j + half, j] = -1.0   # RT[i, i-64] = -1 for i >= 64
        rt[j, j + half] = 1.0    # RT[i, i+64] = +1 for i < 64
    ident = np.eye(128, dtype=np.float32)
    ones = np.ones((128, 128), dtype=np.float32)
    return rt, ident, ones


def shard_inputs(hidden_states, cos, sin, Wq, bq, Wk, bk, Wv, bv, Wo, S=S_FULL,
                 dt="f32r"):
    rt, ident, ones = make_host_constants()
    if dt == "bf16":
        import ml_dtypes
        big = ml_dtypes.bfloat16
    else:
        big = np.float32
    in_maps = []
    for c in range(8):
        b, t = c // TP, c % TP
        m = {
            "hsT": np.ascontiguousarray(hidden_states[b].T).astype(big),
            "cosT": np.ascontiguousarray(cos[b].T).astype(big),
            "sinT": np.ascontiguousarray(sin[b].T).astype(big),
            "wq": np.ascontiguousarray(Wq[:, t * FQ:(t + 1) * FQ]).astype(big),
            "bq": np.ascontiguousarray(bq[t * FQ:(t + 1) * FQ].reshape(QH, HD)),
            "bk": np.ascontiguousarray(bk[t * FKV:(t + 1) * FKV].reshape(KVH, HD)),
            "bv": np.ascontiguousarray(bv[t * FKV:(t + 1) * FKV].reshape(KVH, HD)),
            "wk": np.ascontiguousarray(Wk[:, t * FKV:(t + 1) * FKV]).astype(big),
            "wv": np.ascontiguousarray(Wv[:, t * FKV:(t + 1) * FKV]).astype(big),
            "wo": np.ascontiguousarray(Wo[t * FQ:(t + 1) * FQ, :]).astype(big),
            "rt": rt.astype(big), "ident": ident.astype(big), "ones": ones.astype(big),
        }
        in_maps.append(m)
    return in_maps


_nc_cache = {}


def kernel(hidden_states, cos, sin, Wq, bq, Wk, bk, Wv, bv, Wo):
    global last_exec_time_ns
    from concourse.bass_utils import run_bass_kernel_spmd

    hidden_states = np.asarray(hidden_states, dtype=np.float32)
    cos = np.asarray(cos, dtype=np.float32)
    sin = np.asarray(sin, dtype=np.float32)
    S = hidden_states.shape[1]
    dt = os.environ.get("ATTN_DT", "f32r")
    if (S, dt) not in _nc_cache:
        _nc_cache[(S, dt)] = build_nc(S, dt)
    nc = _nc_cache[(S, dt)]
    in_maps = shard_inputs(hidden_states, cos, sin,
                           np.asarray(Wq, np.float32), np.asarray(bq, np.float32),
                           np.asarray(Wk, np.float32), np.asarray(bk, np.float32),
                           np.asarray(Wv, np.float32), np.asarray(bv, np.float32),
                           np.asarray(Wo, np.float32), S=S, dt=dt)
    trace = bool(int(os.environ.get("ATTN_TRACE", "0")))
    r = run_bass_kernel_spmd(nc, in_maps, list(range(8)), trace=trace)
    last_exec_time_ns = r.exec_time_ns
    outs = [r.results[c]["out"] for c in range(8)]
    full = np.empty((B, S, HID), dtype=np.float32)
    for b in range(B):
        full[b] = outs[b * TP]
        for t in range(1, TP):
            full[b] += outs[b * TP + t]
    return full
